# revision 28
# baseline (speedup 1.0000x reference)
"""Trainium2 Bass kernel for nn_AsymmetricLossCustom (8 NeuronCores).

Math (reference):
    s  = sigmoid(x)
    t  = min(1 - s + 0.05, 1)
    loss = y*ln(max(s,eps)) + (1-y)*ln(max(t,eps))        # [B, C]
    active[b,c] = OR_g ( (any_g[b] & ~has_g[b]) & mask_g[c] )
    out = -(loss * where(active, 0.1, 1.0)).sum()

Device scheme (2 ScalarE passes + 3 VectorE passes per element):
    sp = sigmoid(-x)                     # ACT (sigmoid table set)
    t  = min(sp + 0.05, 1)               # DVE tensor_scalar dual-op, fp16 4x
    c  = 1 - sp            ( = s )       # DVE tensor_scalar dual-op, fp16 4x
    w  = y ? c : t                       # DVE copy_predicated (uint8 y mask)
    loss = ln(w)                         # ACT (natural_log set), in-place on
                                         #   w, accum_out => per-row sum(loss)

Sigmoid and ln live in different ACT table sets, so chunks are processed in
two groups: all sigmoids of a group, then all lns - 2 table loads per group
instead of 2 per chunk (a sync=False dep chain pins the ACT emission order
against scheduler interleaving).

The `active` down-weighting only touches columns appearing in one of the
three index arrays (<=170 of 9605). The host gathers those columns, builds
weights avB = 1 + 0.9*active (pure index/mask preprocessing), and APPENDS
the gathered x/y columns to chunk 0 of the main stream (laid out
per-partition to match the main view). The appended elements are counted
twice - once in the plain accumulation (loss_sum' = sum(loss) + sum_g lg)
and once in a single fused scalar_tensor_tensor reduction
(corr2 = sum_g avB*lg), so

    result = -loss_sum' + corr2
           = -sum(loss) + 0.9*sum_g(active*lg)            (exact)

Sharding: pure data parallel over the batch. Each core gets 512 rows,
viewed as [128 partitions, 38420 free] (4 rows per partition, contiguous).
y travels as uint8 (it is exactly 0/1) and doubles as the predication mask.
Host sums the 8 per-core [128, 2] partials:
    result = -sum(out[:,0]) + sum(out[:,1]).
"""

import sys

import numpy as np

if "/opt/trn_rl_repo" not in sys.path:
    sys.path.insert(0, "/opt/trn_rl_repo")

B, C = 4096, 9605
NCORES = 8
ROWS = B // NCORES          # 512 rows per core
P = 128                     # SBUF partitions
RPP = ROWS // P             # 4 rows per partition
FREE = RPP * C              # 38420 f32 per partition
NCHUNK = 10
# Non-uniform chunk sizes (all even, sum = FREE): small chunks first so the
# first sigmoid starts as soon as possible, medium last chunk for the tail.
SIZES = [1280, 2560, 3584] + [4428] * 7
APPEND_CHUNK = 2            # chunk that carries the GU appendix
GROUPS = [range(0, 5), range(5, 10)]
WBUFS = 6                   # covers one phase group
XBUFS = 4
YBUFS = 5
U_PAD = 176                 # padded union-column count (>= 70+70+30)
GU = RPP * U_PAD            # 704 appended free elements on chunk 0
CLIP = 0.05
ALPHA = 0.1

TRACE = False               # set True (e.g. from test.py) to capture an NTFF profile
LAST_RESULTS = None         # BassKernelResults of the most recent run

_NC = None


def _build_program(nchunk=None, groups=None, wbufs=None, xbufs=None,
                   ybufs=None, sizes=None, append_chunk=None,
                   ln_inplace=True, sbufs=3, cbufs=3, ydma_gpsimd=False):
    nchunk = nchunk or NCHUNK
    groups = groups or GROUPS
    wbufs = wbufs or WBUFS
    xbufs = xbufs or XBUFS
    ybufs = ybufs or YBUFS
    sizes = sizes or SIZES
    append_chunk = APPEND_CHUNK if append_chunk is None else append_chunk
    offs = [0]
    for sz in sizes:
        offs.append(offs[-1] + sz)
    assert offs[-1] == FREE

    import concourse.bacc as bacc
    import concourse.mybir as mybir
    from concourse import tile
    from concourse.tile import add_dep_helper

    f32 = mybir.dt.float32
    f16 = mybir.dt.float16
    u8 = mybir.dt.uint8
    Alu = mybir.AluOpType
    Act = mybir.ActivationFunctionType
    AX = mybir.AxisListType

    # Force the ACT engine to execute activations in emission order -
    # otherwise the Tile scheduler interleaves sigmoid and ln chunks and
    # the compiler inserts an ACT_TABLE_LOAD (~1.3us) before nearly every
    # activation instead of one per phase.
    _prev_act = [None]

    def act_order(bi):
        if _prev_act[0] is not None:
            add_dep_helper(bi.ins, _prev_act[0].ins, sync=False,
                           reason="act table-set phase order")
        _prev_act[0] = bi
        return bi

    nc = bacc.Bacc(
        "TRN2",
        target_bir_lowering=False,
        debug=False,
        enable_asserts=False,
        num_devices=NCORES,
    )

    x = nc.dram_tensor("x", [P, FREE], f32, kind="ExternalInput").ap()
    y = nc.dram_tensor("y", [P, FREE], u8, kind="ExternalInput").ap()
    xga = nc.dram_tensor("xga", [P, GU], f32, kind="ExternalInput").ap()
    yga = nc.dram_tensor("yga", [P, GU], u8, kind="ExternalInput").ap()
    avb = nc.dram_tensor("avb", [P, GU], f32, kind="ExternalInput").ap()
    out = nc.dram_tensor("out", [P, 2], f32, kind="ExternalOutput").ap()

    with tile.TileContext(nc) as tc:
        with (
            tc.tile_pool(name="xp", bufs=xbufs) as xp,
            tc.tile_pool(name="yp", bufs=ybufs) as yp,
            tc.tile_pool(name="sp", bufs=sbufs) as sp,
            tc.tile_pool(name="wp", bufs=wbufs) as wp,
            tc.tile_pool(name="cp", bufs=cbufs) as cp,
            tc.tile_pool(name="lp", bufs=2) as lp,
            tc.tile_pool(name="accp", bufs=1) as accp,
            tc.tile_pool(name="finp", bufs=1) as finp,
        ):
            accLW = accp.tile([P, nchunk], f32, tag="accLW")
            accC = accp.tile([P, 1], f32, tag="accC")
            avbt = accp.tile([P, GU], f32, tag="avbt")
            nc.sync.dma_start(avbt[:], avb[:])

            for gi, grp in enumerate(groups):
                # ---- DMA + sigmoid phase -------------------------------
                yts, sts, width = {}, {}, {}
                for k in grp:
                    fk = sizes[k]
                    fw = fk + GU if k == append_chunk else fk
                    cs = slice(offs[k], offs[k + 1])
                    xt = xp.tile([P, fw], f32, tag="x")
                    nc.sync.dma_start(xt[:, 0:fk], x[:, cs])
                    yt = yp.tile([P, fw], u8, tag="y")
                    yeng = nc.gpsimd if ydma_gpsimd else nc.sync
                    yeng.dma_start(yt[:, 0:fk], y[:, cs])
                    if k == append_chunk:
                        nc.sync.dma_start(xt[:, fk:fw], xga[:])
                        yeng.dma_start(yt[:, fk:fw], yga[:])
                    st = sp.tile([P, fw], f16, tag="s")
                    act_order(nc.scalar.activation(st[:], xt[:], Act.Sigmoid,
                                                   scale=-1.0))
                    yts[k], sts[k], width[k] = yt, st, fw

                # ---- blend phase (DVE) ---------------------------------
                wts = {}
                for k in grp:
                    st, yt, fw = sts[k], yts[k], width[k]
                    wt = wp.tile([P, fw], f16, tag="w")
                    nc.vector.tensor_scalar(wt[:], st[:], CLIP, 1.0,
                                            Alu.add, Alu.min)
                    ct = cp.tile([P, fw], f16, tag="c")
                    nc.vector.tensor_scalar(ct[:], st[:], -1.0, 1.0,
                                            Alu.mult, Alu.add)
                    nc.vector.copy_predicated(wt[:], yt[:], ct[:])
                    wts[k] = wt

                # ---- Ln phase ------------------------------------------
                for k in grp:
                    wt, fw = wts[k], width[k]
                    lt = wt if ln_inplace else lp.tile([P, fw], f16, tag="lt")
                    act_order(nc.scalar.activation(
                        lt[:], wt[:], Act.Ln, accum_out=accLW[:, k : k + 1]))
                    if k == append_chunk:
                        # corr2 = sum(avB * lg) over the appended region
                        ja = lp.tile([P, GU], f16, tag="ja")
                        nc.vector.scalar_tensor_tensor(
                            ja[:], avbt[:], 0.0, lt[:, sizes[k]:fw],
                            Alu.bypass, Alu.mult,
                            accum_out=accC[:, 0:1],
                        )

            # ---- final combine -> out [P, 2] ---------------------------
            lossr = finp.tile([P, 1], f32, tag="lossr")
            nc.vector.tensor_reduce(lossr[:], accLW[:], AX.X, Alu.add)

            osb = finp.tile([P, 2], f32, tag="osb")
            nc.vector.tensor_copy(out=osb[:, 0:1], in_=lossr[:])
            nc.vector.tensor_copy(out=osb[:, 1:2], in_=accC[:])
            nc.sync.dma_start(out[:], osb[:])

    nc.compile()
    return nc


def _get_nc():
    global _NC
    if _NC is None:
        _NC = _build_program()
    return _NC


def _ensure_ntff_hook():
    """Register the axon NTFF profile hook if the image's antenv lacks it."""
    import contextlib
    import ctypes
    import types

    try:
        from antenv.axon_hooks import get_axon_ntff_profile_hook  # noqa: F401
        return
    except ImportError:
        pass

    so_path = "/opt/axon/libaxon_pjrt.so"
    try:
        lib = ctypes.CDLL(so_path)
    except OSError:
        return
    if not hasattr(lib, "axon_start_nrt_profile"):
        return
    lib.axon_start_nrt_profile.argtypes = [
        ctypes.POINTER(ctypes.c_int64),
        ctypes.c_size_t,
    ]
    lib.axon_start_nrt_profile.restype = ctypes.c_int64
    lib.axon_stop_nrt_profile.argtypes = [ctypes.c_char_p]
    lib.axon_stop_nrt_profile.restype = ctypes.c_int64

    @contextlib.contextmanager
    def _hook(output_dir, device_ids):
        import jax

        jax.devices()
        if device_ids:
            ids = (ctypes.c_int64 * len(device_ids))(*device_ids)
            rc = lib.axon_start_nrt_profile(ids, len(device_ids))
        else:
            rc = lib.axon_start_nrt_profile(None, 0)
        if rc != 0:
            raise RuntimeError(f"axon_start_nrt_profile rc={rc}")
        try:
            yield
        finally:
            n = lib.axon_stop_nrt_profile(str(output_dir).encode())
            print(f"ntff profile: {n} file(s) written to {output_dir}",
                  file=sys.stderr)

    mod = types.ModuleType("antenv.axon_hooks")
    mod.get_axon_ntff_profile_hook = lambda: _hook
    mod.set_axon_ntff_profile_hook = lambda h: None
    sys.modules["antenv.axon_hooks"] = mod


def _prepare_inputs(x, y, recycle_ind, donate_ind, compost_ind):
    """Host-side sharding and index preprocessing -> per-core in_maps."""
    x = np.ascontiguousarray(x, dtype=np.float32)
    y = np.ascontiguousarray(y, dtype=np.float32)
    yu8 = y.astype(np.uint8)
    recycle_ind = np.asarray(recycle_ind).astype(np.int64)
    donate_ind = np.asarray(donate_ind).astype(np.int64)
    compost_ind = np.asarray(compost_ind).astype(np.int64)

    # Union of group columns, padded to the fixed program width. Pad
    # columns get avB = 1 so their (doubly counted) contribution cancels.
    cols = np.unique(np.concatenate([recycle_ind, donate_ind, compost_ind]))
    u = len(cols)
    assert u <= U_PAD, (u, U_PAD)
    colsp = np.concatenate([cols, np.zeros(U_PAD - u, dtype=cols.dtype)])

    def mask_v(ind):
        v = np.zeros(U_PAD, np.float32)
        v[:u] = np.isin(cols, ind).astype(np.float32)
        return v

    mrv = mask_v(recycle_ind)
    mdv = mask_v(donate_ind)
    mcv = mask_v(compost_ind)

    xg = np.ascontiguousarray(x[:, colsp])          # [B, U_PAD]
    ygf = y[:, colsp]
    yg8 = np.ascontiguousarray(yu8[:, colsp])

    # active[b, j] from the group masks and per-row has-group flags
    has_r = (ygf * mrv).sum(axis=1) > 0
    has_d = (ygf * mdv).sum(axis=1) > 0
    has_c = (ygf * mcv).sum(axis=1) > 0
    any_g = has_r | has_d | has_c
    a_r = (any_g & ~has_r).astype(np.float32)
    a_d = (any_g & ~has_d).astype(np.float32)
    a_c = (any_g & ~has_c).astype(np.float32)
    av = np.minimum(a_r[:, None] * mrv + a_d[:, None] * mdv
                    + a_c[:, None] * mcv, 1.0)
    avb = (1.0 + (1.0 - ALPHA) * av).astype(np.float32)  # [B, U_PAD]

    in_maps = []
    for i in range(NCORES):
        rs = slice(i * ROWS, (i + 1) * ROWS)
        in_maps.append({
            "x": x[rs].reshape(P, FREE),
            "y": yu8[rs].reshape(P, FREE),
            "xga": xg[rs].reshape(P, GU),
            "yga": yg8[rs].reshape(P, GU),
            "avb": avb[rs].reshape(P, GU),
        })
    return in_maps


def kernel(x, y, recycle_ind, donate_ind, compost_ind):
    global LAST_RESULTS
    import concourse.bass_utils as bass_utils

    # Avoid any network artifact upload in the (optional) trace path.
    bass_utils.upload_artifacts = lambda tmpdir: "local://" + tmpdir
    _ensure_ntff_hook()

    in_maps = _prepare_inputs(x, y, recycle_ind, donate_ind, compost_ind)
    nc = _get_nc()

    res = bass_utils.run_bass_kernel_spmd(
        nc, in_maps, core_ids=list(range(NCORES)), trace=TRACE
    )
    LAST_RESULTS = res

    loss_sum = 0.0
    corr2 = 0.0
    for r in res.results:
        o = r["out"].astype(np.float64)
        loss_sum += o[:, 0].sum()
        corr2 += o[:, 1].sum()

    total = -loss_sum + corr2
    return np.asarray(total, dtype=np.float32)


# revision 30
# speedup vs baseline: 1.0128x; 1.0128x over previous
"""Trainium2 Bass kernel for nn_AsymmetricLossCustom (8 NeuronCores).

Math (reference):
    s  = sigmoid(x)
    t  = min(1 - s + 0.05, 1)
    loss = y*ln(max(s,eps)) + (1-y)*ln(max(t,eps))        # [B, C]
    active[b,c] = OR_g ( (any_g[b] & ~has_g[b]) & mask_g[c] )
    out = -(loss * where(active, 0.1, 1.0)).sum()

Device scheme (2 ScalarE passes + 3 VectorE passes per element):
    sp = sigmoid(-x)                     # ACT (sigmoid table set)
    t  = min(sp + 0.05, 1)               # DVE tensor_scalar dual-op, fp16 4x
    c  = 1 - sp            ( = s )       # DVE tensor_scalar dual-op, fp16 4x
    w  = y ? c : t                       # DVE copy_predicated (uint8 y mask)
    loss = ln(w)                         # ACT (natural_log set), in-place on
                                         #   w, accum_out => per-row sum(loss)

Sigmoid and ln live in different ACT table sets, so chunks are processed in
two groups: all sigmoids of a group, then all lns - 2 table loads per group
instead of 2 per chunk (a sync=False dep chain pins the ACT emission order
against scheduler interleaving).

The `active` down-weighting only touches columns appearing in one of the
three index arrays (<=170 of 9605). The host gathers those columns, builds
weights avB = 1 + 0.9*active (pure index/mask preprocessing), and APPENDS
the gathered x/y columns to chunk 0 of the main stream (laid out
per-partition to match the main view). The appended elements are counted
twice - once in the plain accumulation (loss_sum' = sum(loss) + sum_g lg)
and once in a single fused scalar_tensor_tensor reduction
(corr2 = sum_g avB*lg), so

    result = -loss_sum' + corr2
           = -sum(loss) + 0.9*sum_g(active*lg)            (exact)

Sharding: pure data parallel over the batch. Each core gets 512 rows,
viewed as [128 partitions, 38420 free] (4 rows per partition, contiguous).
y travels as uint8 (it is exactly 0/1) and doubles as the predication mask.
Host sums the 8 per-core [128, 2] partials:
    result = -sum(out[:,0]) + sum(out[:,1]).
"""

import sys

import numpy as np

if "/opt/trn_rl_repo" not in sys.path:
    sys.path.insert(0, "/opt/trn_rl_repo")

B, C = 4096, 9605
NCORES = 8
ROWS = B // NCORES          # 512 rows per core
P = 128                     # SBUF partitions
RPP = ROWS // P             # 4 rows per partition
FREE = RPP * C              # 38420 f32 per partition
NCHUNK = 10
# Non-uniform chunk sizes (all even, sum = FREE): small chunks first so the
# first sigmoid starts as soon as possible, medium last chunk for the tail.
SIZES = [1280, 2560, 3584] + [4428] * 7
APPEND_CHUNK = 2            # chunk that carries the GU appendix
GROUPS = [range(0, 5), range(5, 10)]
WBUFS = 6                   # covers one phase group
XBUFS = 4
YBUFS = 5
U_PAD = 176                 # padded union-column count (>= 70+70+30)
GU = RPP * U_PAD            # 704 appended free elements on chunk 0
CLIP = 0.05
ALPHA = 0.1

TRACE = False               # set True (e.g. from test.py) to capture an NTFF profile
LAST_RESULTS = None         # BassKernelResults of the most recent run

_NC = None


def _build_program(nchunk=None, groups=None, wbufs=None, xbufs=None,
                   ybufs=None, sizes=None, append_chunk=None,
                   ln_inplace=True, sbufs=3, cbufs=3, ydma_gpsimd=False,
                   avbt_late=True):
    nchunk = nchunk or NCHUNK
    groups = groups or GROUPS
    wbufs = wbufs or WBUFS
    xbufs = xbufs or XBUFS
    ybufs = ybufs or YBUFS
    sizes = sizes or SIZES
    append_chunk = APPEND_CHUNK if append_chunk is None else append_chunk
    offs = [0]
    for sz in sizes:
        offs.append(offs[-1] + sz)
    assert offs[-1] == FREE

    import concourse.bacc as bacc
    import concourse.mybir as mybir
    from concourse import tile
    from concourse.tile import add_dep_helper

    f32 = mybir.dt.float32
    f16 = mybir.dt.float16
    u8 = mybir.dt.uint8
    Alu = mybir.AluOpType
    Act = mybir.ActivationFunctionType
    AX = mybir.AxisListType

    # Force the ACT engine to execute activations in emission order -
    # otherwise the Tile scheduler interleaves sigmoid and ln chunks and
    # the compiler inserts an ACT_TABLE_LOAD (~1.3us) before nearly every
    # activation instead of one per phase.
    _prev_act = [None]

    def act_order(bi):
        if _prev_act[0] is not None:
            add_dep_helper(bi.ins, _prev_act[0].ins, sync=False,
                           reason="act table-set phase order")
        _prev_act[0] = bi
        return bi

    nc = bacc.Bacc(
        "TRN2",
        target_bir_lowering=False,
        debug=False,
        enable_asserts=False,
        num_devices=NCORES,
    )

    x = nc.dram_tensor("x", [P, FREE], f32, kind="ExternalInput").ap()
    y = nc.dram_tensor("y", [P, FREE], u8, kind="ExternalInput").ap()
    xga = nc.dram_tensor("xga", [P, GU], f32, kind="ExternalInput").ap()
    yga = nc.dram_tensor("yga", [P, GU], u8, kind="ExternalInput").ap()
    avb = nc.dram_tensor("avb", [P, GU], f32, kind="ExternalInput").ap()
    out = nc.dram_tensor("out", [P, 2], f32, kind="ExternalOutput").ap()

    with tile.TileContext(nc) as tc:
        with (
            tc.tile_pool(name="xp", bufs=xbufs) as xp,
            tc.tile_pool(name="yp", bufs=ybufs) as yp,
            tc.tile_pool(name="sp", bufs=sbufs) as sp,
            tc.tile_pool(name="wp", bufs=wbufs) as wp,
            tc.tile_pool(name="cp", bufs=cbufs) as cp,
            tc.tile_pool(name="lp", bufs=2) as lp,
            tc.tile_pool(name="accp", bufs=1) as accp,
            tc.tile_pool(name="finp", bufs=1) as finp,
        ):
            accLW = accp.tile([P, nchunk], f32, tag="accLW")
            accC = accp.tile([P, 1], f32, tag="accC")
            avbt = accp.tile([P, GU], f32, tag="avbt")
            if not avbt_late:
                nc.sync.dma_start(avbt[:], avb[:])

            for gi, grp in enumerate(groups):
                # ---- DMA + sigmoid phase -------------------------------
                yts, sts, width = {}, {}, {}
                for k in grp:
                    fk = sizes[k]
                    fw = fk + GU if k == append_chunk else fk
                    cs = slice(offs[k], offs[k + 1])
                    xt = xp.tile([P, fw], f32, tag="x")
                    nc.sync.dma_start(xt[:, 0:fk], x[:, cs])
                    yt = yp.tile([P, fw], u8, tag="y")
                    yeng = nc.gpsimd if ydma_gpsimd else nc.sync
                    yeng.dma_start(yt[:, 0:fk], y[:, cs])
                    if k == append_chunk:
                        nc.sync.dma_start(xt[:, fk:fw], xga[:])
                        yeng.dma_start(yt[:, fk:fw], yga[:])
                    st = sp.tile([P, fw], f16, tag="s")
                    act_order(nc.scalar.activation(st[:], xt[:], Act.Sigmoid,
                                                   scale=-1.0))
                    yts[k], sts[k], width[k] = yt, st, fw

                # ---- blend phase (DVE) ---------------------------------
                if gi == 0 and avbt_late:
                    nc.sync.dma_start(avbt[:], avb[:])
                wts = {}
                for k in grp:
                    st, yt, fw = sts[k], yts[k], width[k]
                    wt = wp.tile([P, fw], f16, tag="w")
                    nc.vector.tensor_scalar(wt[:], st[:], CLIP, 1.0,
                                            Alu.add, Alu.min)
                    ct = cp.tile([P, fw], f16, tag="c")
                    nc.vector.tensor_scalar(ct[:], st[:], -1.0, 1.0,
                                            Alu.mult, Alu.add)
                    nc.vector.copy_predicated(wt[:], yt[:], ct[:])
                    wts[k] = wt

                # ---- Ln phase ------------------------------------------
                for k in grp:
                    wt, fw = wts[k], width[k]
                    lt = wt if ln_inplace else lp.tile([P, fw], f16, tag="lt")
                    act_order(nc.scalar.activation(
                        lt[:], wt[:], Act.Ln, accum_out=accLW[:, k : k + 1]))
                    if k == append_chunk:
                        # corr2 = sum(avB * lg) over the appended region
                        ja = lp.tile([P, GU], f16, tag="ja")
                        nc.vector.scalar_tensor_tensor(
                            ja[:], avbt[:], 0.0, lt[:, sizes[k]:fw],
                            Alu.bypass, Alu.mult,
                            accum_out=accC[:, 0:1],
                        )

            # ---- final combine -> out [P, 2] ---------------------------
            lossr = finp.tile([P, 1], f32, tag="lossr")
            nc.vector.tensor_reduce(lossr[:], accLW[:], AX.X, Alu.add)

            osb = finp.tile([P, 2], f32, tag="osb")
            nc.vector.tensor_copy(out=osb[:, 0:1], in_=lossr[:])
            nc.vector.tensor_copy(out=osb[:, 1:2], in_=accC[:])
            nc.sync.dma_start(out[:], osb[:])

    nc.compile()
    return nc


def _get_nc():
    global _NC
    if _NC is None:
        _NC = _build_program()
    return _NC


def _ensure_ntff_hook():
    """Register the axon NTFF profile hook if the image's antenv lacks it."""
    import contextlib
    import ctypes
    import types

    try:
        from antenv.axon_hooks import get_axon_ntff_profile_hook  # noqa: F401
        return
    except ImportError:
        pass

    so_path = "/opt/axon/libaxon_pjrt.so"
    try:
        lib = ctypes.CDLL(so_path)
    except OSError:
        return
    if not hasattr(lib, "axon_start_nrt_profile"):
        return
    lib.axon_start_nrt_profile.argtypes = [
        ctypes.POINTER(ctypes.c_int64),
        ctypes.c_size_t,
    ]
    lib.axon_start_nrt_profile.restype = ctypes.c_int64
    lib.axon_stop_nrt_profile.argtypes = [ctypes.c_char_p]
    lib.axon_stop_nrt_profile.restype = ctypes.c_int64

    @contextlib.contextmanager
    def _hook(output_dir, device_ids):
        import jax

        jax.devices()
        if device_ids:
            ids = (ctypes.c_int64 * len(device_ids))(*device_ids)
            rc = lib.axon_start_nrt_profile(ids, len(device_ids))
        else:
            rc = lib.axon_start_nrt_profile(None, 0)
        if rc != 0:
            raise RuntimeError(f"axon_start_nrt_profile rc={rc}")
        try:
            yield
        finally:
            n = lib.axon_stop_nrt_profile(str(output_dir).encode())
            print(f"ntff profile: {n} file(s) written to {output_dir}",
                  file=sys.stderr)

    mod = types.ModuleType("antenv.axon_hooks")
    mod.get_axon_ntff_profile_hook = lambda: _hook
    mod.set_axon_ntff_profile_hook = lambda h: None
    sys.modules["antenv.axon_hooks"] = mod


def _prepare_inputs(x, y, recycle_ind, donate_ind, compost_ind):
    """Host-side sharding and index preprocessing -> per-core in_maps."""
    x = np.ascontiguousarray(x, dtype=np.float32)
    y = np.ascontiguousarray(y, dtype=np.float32)
    yu8 = y.astype(np.uint8)
    recycle_ind = np.asarray(recycle_ind).astype(np.int64)
    donate_ind = np.asarray(donate_ind).astype(np.int64)
    compost_ind = np.asarray(compost_ind).astype(np.int64)

    # Union of group columns, padded to the fixed program width. Pad
    # columns get avB = 1 so their (doubly counted) contribution cancels.
    cols = np.unique(np.concatenate([recycle_ind, donate_ind, compost_ind]))
    u = len(cols)
    assert u <= U_PAD, (u, U_PAD)
    colsp = np.concatenate([cols, np.zeros(U_PAD - u, dtype=cols.dtype)])

    def mask_v(ind):
        v = np.zeros(U_PAD, np.float32)
        v[:u] = np.isin(cols, ind).astype(np.float32)
        return v

    mrv = mask_v(recycle_ind)
    mdv = mask_v(donate_ind)
    mcv = mask_v(compost_ind)

    xg = np.ascontiguousarray(x[:, colsp])          # [B, U_PAD]
    ygf = y[:, colsp]
    yg8 = np.ascontiguousarray(yu8[:, colsp])

    # active[b, j] from the group masks and per-row has-group flags
    has_r = (ygf * mrv).sum(axis=1) > 0
    has_d = (ygf * mdv).sum(axis=1) > 0
    has_c = (ygf * mcv).sum(axis=1) > 0
    any_g = has_r | has_d | has_c
    a_r = (any_g & ~has_r).astype(np.float32)
    a_d = (any_g & ~has_d).astype(np.float32)
    a_c = (any_g & ~has_c).astype(np.float32)
    av = np.minimum(a_r[:, None] * mrv + a_d[:, None] * mdv
                    + a_c[:, None] * mcv, 1.0)
    avb = (1.0 + (1.0 - ALPHA) * av).astype(np.float32)  # [B, U_PAD]

    in_maps = []
    for i in range(NCORES):
        rs = slice(i * ROWS, (i + 1) * ROWS)
        in_maps.append({
            "x": x[rs].reshape(P, FREE),
            "y": yu8[rs].reshape(P, FREE),
            "xga": xg[rs].reshape(P, GU),
            "yga": yg8[rs].reshape(P, GU),
            "avb": avb[rs].reshape(P, GU),
        })
    return in_maps


def kernel(x, y, recycle_ind, donate_ind, compost_ind):
    global LAST_RESULTS
    import concourse.bass_utils as bass_utils

    # Avoid any network artifact upload in the (optional) trace path.
    bass_utils.upload_artifacts = lambda tmpdir: "local://" + tmpdir
    _ensure_ntff_hook()

    in_maps = _prepare_inputs(x, y, recycle_ind, donate_ind, compost_ind)
    nc = _get_nc()

    res = bass_utils.run_bass_kernel_spmd(
        nc, in_maps, core_ids=list(range(NCORES)), trace=TRACE
    )
    LAST_RESULTS = res

    loss_sum = 0.0
    corr2 = 0.0
    for r in res.results:
        o = r["out"].astype(np.float64)
        loss_sum += o[:, 0].sum()
        corr2 += o[:, 1].sum()

    total = -loss_sum + corr2
    return np.asarray(total, dtype=np.float32)
